# revision 14
# baseline (speedup 1.0000x reference)
"""CLS-AttentionPool2d Trainium2 kernel (8 NeuronCores, data-parallel over batch).

Math refactoring (single CLS query => tiny attention):
  tokens[j] = x[b,:,j]                         (j = 0..1023, native [C, HW] layout)
  mean      = tokens.mean(j);  cls = mean + pos0
  q  = (Wq @ cls + bq) / sqrt(C)
  qblk[k, (s,h)] = q_s[k] * [head(k) == h]     (block-diag arrangement)
  m  = Wk.T @ qblk                             # m[c, slot] per-head key-projected query
  scores[slot, j]    = m.T @ x_tokens  (+ KP-term for pos_emb, KP = Wk @ pos.T host-precomputed)
  scores[slot, cls]  = rowmean(token scores) + KP-cls-term     (mean is linear)
  p = softmax(scores)  ;  p' = p_tok + p_cls/1024  (folds CLS-mean into token weights)
  w  = p'.T @ (tokens via PE-transpose) + p'.T @ pos_tok + p_cls * pos0adj
  out = Wv @ w + bv                            (per-head block of Wv)

The q.bk term is constant across j => dropped (softmax shift invariance).
"""

import math
import numpy as np

import concourse.bass as bass
import concourse.mybir as mybir
import concourse.tile as tile
from concourse import bacc
from concourse.bass import ts
from concourse.bass_utils import run_bass_kernel_spmd

F32 = mybir.dt.float32
BF16 = mybir.dt.bfloat16
AX = mybir.AxisListType
ALU = mybir.AluOpType
ACTF = mybir.ActivationFunctionType

B, C, HW = 64, 512, 1024
NH, DH = 8, 64
NCORES = 8
BPC = B // NCORES          # 8 batches per core
GRP = 4                    # batches per group (2 groups per core)
NGRP = BPC // GRP
CT = C // 128              # 4 c-chunks
JT = HW // 128             # 8 j-chunks
ISQ = 1.0 / math.sqrt(C)
XBAR_SLOTS = (0, 1, 2, 3)

_CACHE = {}


def _build_nc():
    nc = bacc.Bacc("TRN2", target_bir_lowering=False, debug=False,
                   num_devices=NCORES)

    # ---- DRAM I/O ----
    xs = nc.dram_tensor("xs", [BPC, C, HW], F32, kind="ExternalInput")
    wqt = nc.dram_tensor("wqt", [128, CT, C], F32, kind="ExternalInput")
    wk = nc.dram_tensor("wk", [128, CT, C], BF16, kind="ExternalInput")
    wvt = nc.dram_tensor("wvt", [128, CT, C], F32, kind="ExternalInput")
    kp = nc.dram_tensor("kp", [128, CT, HW + 1], BF16, kind="ExternalInput")
    postok = nc.dram_tensor("postok", [128, JT, C], BF16, kind="ExternalInput")
    pos0row = nc.dram_tensor("pos0row", [1, C], BF16, kind="ExternalInput")
    pos0 = nc.dram_tensor("pos0", [128, CT], F32, kind="ExternalInput")
    bqs = nc.dram_tensor("bqs", [128, CT], F32, kind="ExternalInput")
    bv = nc.dram_tensor("bv", [128, CT], F32, kind="ExternalInput")
    mask32 = nc.dram_tensor("mask32", [128, CT, 32], F32, kind="ExternalInput")
    ident = nc.dram_tensor("ident", [128, 128], BF16, kind="ExternalInput")
    identf = nc.dram_tensor("identf", [128, 128], F32, kind="ExternalInput")
    out_d = nc.dram_tensor("out", [BPC, C], F32, kind="ExternalOutput")

    with tile.TileContext(nc) as tc:
        with (
            tc.tile_pool(name="persist", bufs=1) as pp,
            tc.tile_pool(name="big", bufs=10) as bigp,
            tc.tile_pool(name="work", bufs=2) as wp,
            tc.tile_pool(name="psA", bufs=1, space="PSUM") as psA,
            tc.tile_pool(name="psB", bufs=2, space="PSUM") as psB,
            tc.tile_pool(name="psC", bufs=2, space="PSUM") as psC,
            tc.tile_pool(name="psD", bufs=1, space="PSUM") as psD,
        ):
            # ---- persistent loads ----
            wqt_s = pp.tile([128, CT, C], F32)
            wk_s = pp.tile([128, CT, C], BF16)
            wvt_s = pp.tile([128, CT, C], F32)
            kp_s = pp.tile([128, CT, HW + 1], BF16)
            postok_s = pp.tile([128, JT, C], BF16)
            pos0row_s = pp.tile([1, C], BF16)
            pos0_s = pp.tile([128, CT], F32)
            bqs_s = pp.tile([128, CT], F32)
            bv_s = pp.tile([128, CT], F32)
            mask_s = pp.tile([128, CT, 32], F32)
            ident_s = pp.tile([128, 128], BF16)
            identf_s = pp.tile([128, 128], F32)
            for dst, src in [(wqt_s, wqt), (wk_s, wk), (wvt_s, wvt),
                             (kp_s, kp), (postok_s, postok),
                             (pos0row_s, pos0row), (pos0_s, pos0),
                             (bqs_s, bqs), (bv_s, bv), (mask_s, mask32),
                             (ident_s, ident), (identf_s, identf)]:
                nc.sync.dma_start(out=dst[:], in_=src[:])

            for g in range(NGRP):
                # ---------------- phase A: load + means ----------------
                xb = []
                means = wp.tile([128, CT, GRP], F32, tag="means")
                junk = wp.tile([128, HW], BF16, tag="junk")
                for s in range(GRP):
                    xt = bigp.tile([128, CT, HW], BF16, tag="big")
                    xb.append(xt)
                    nc.gpsimd.dma_start(
                        out=xt[:],
                        in_=xs[g * GRP + s].rearrange("(p t) j -> p t j", t=CT))
                    if s < 2:
                        for t in range(CT):
                            nc.scalar.activation(
                                junk[:], xt[:, t, :], ACTF.Copy,
                                scale=1.0 / HW,
                                accum_out=means[:, t, s:s + 1])
                    else:
                        nc.vector.tensor_reduce(
                            means[:, :, s:s + 1].rearrange("p t one -> p (t one)"),
                            xt[:], axis=AX.X, op=ALU.add)
                        nc.vector.tensor_scalar_mul(
                            means[:, :, s:s + 1].rearrange("p t one -> p (t one)"),
                            means[:, :, s:s + 1].rearrange("p t one -> p (t one)"),
                            1.0 / HW)

                # ---------------- phase B: cls -> q -> qblk -> m ----------------
                cls_all = wp.tile([128, CT, GRP], F32, tag="cls")
                nc.vector.tensor_add(
                    cls_all[:], means[:],
                    pos0_s[:, :, None].broadcast_to([128, CT, GRP]))

                q_ps = psC.tile([128, CT, GRP], F32, tag="psC")
                for mc in range(CT):
                    for tk in range(CT):
                        nc.tensor.matmul(
                            q_ps[:, mc, :], wqt_s[:, tk, ts(mc, 128)],
                            cls_all[:, tk, :],
                            start=(tk == 0), stop=(tk == CT - 1))
                q_sb = wp.tile([128, CT, GRP], F32, tag="qsb")
                nc.vector.scalar_tensor_tensor(
                    out=q_sb[:], in0=q_ps[:], scalar=ISQ,
                    in1=bqs_s[:, :, None].broadcast_to([128, CT, GRP]),
                    op0=ALU.mult, op1=ALU.add)

                qblk = wp.tile([128, CT, GRP, 32], BF16, tag="qblk")
                nc.vector.tensor_mul(
                    qblk[:],
                    q_sb[:, :, :, None].broadcast_to([128, CT, GRP, 32]),
                    mask_s[:, :, None, :].broadcast_to([128, CT, GRP, 32]))
                qblk_f = qblk[:].rearrange("p t s u -> p t (s u)")

                m_ps = psC.tile([128, CT, 128], F32, tag="psC")
                for mc in range(CT):
                    for tk in range(CT):
                        nc.tensor.matmul(
                            m_ps[:, mc, :], wk_s[:, tk, ts(mc, 128)],
                            qblk_f[:, tk, :],
                            start=(tk == 0), stop=(tk == CT - 1))
                m_sb = wp.tile([128, CT, 128], BF16, tag="msb")
                nc.vector.tensor_copy(m_sb[:], m_ps[:])

                # ---------------- scores ----------------
                sc_ps = psA.tile([128, 3, 512], F32, tag="psA")
                for bk2, (j0, wdt) in enumerate(((0, 512), (512, 512), (1024, 1))):
                    for tk in range(CT):
                        nc.tensor.matmul(
                            sc_ps[:, bk2, 0:wdt], qblk_f[:, tk, :],
                            kp_s[:, tk, j0:j0 + wdt],
                            start=(tk == 0), stop=False,
                            skip_group_check=True)
                # token scores accumulate on top; transpose after to free xb
                toks = []
                for s in range(GRP):
                    for tk in range(CT):
                        for jc in range(2):
                            nc.tensor.matmul(
                                sc_ps[32 * s:32 * s + 8, jc, :],
                                m_sb[:, tk, 32 * s:32 * s + 8],
                                xb[s][:, tk, ts(jc, 512)],
                                start=False, stop=False,
                                tile_position=(0, 32 * s),
                                skip_group_check=True)
                    # transpose x -> tokens [j, (t, c')] (frees xb[s]):
                    # xbar DMA for some slots, PE transpose for others
                    tok = bigp.tile([128, CT, JT, 128], BF16, tag="big")
                    toks.append(tok)
                    if s in XBAR_SLOTS:
                        nc.sync.dma_start_transpose(
                            tok[:].rearrange("p t j c -> p (t j) c"), xb[s][:])
                    else:
                        for jc in range(JT):
                            tp = psB.tile([128, CT, 128], BF16, tag="psB")
                            for t in range(CT):
                                nc.tensor.transpose(
                                    tp[:, t, :], xb[s][:, t, ts(jc, 128)],
                                    ident_s[:])
                            nc.scalar.copy(tok[:, :, jc, :], tp[:])

                # CLS col: += rowmean of token scores
                redcol = wp.tile([128, 1], F32, tag="redcol")
                nc.vector.reduce_sum(redcol[:], sc_ps[:, 0:2, :], axis=AX.XY)
                nc.vector.scalar_tensor_tensor(
                    out=sc_ps[:, 2, 0:1], in0=redcol[:], scalar=1.0 / HW,
                    in1=sc_ps[:, 2, 0:1], op0=ALU.mult, op1=ALU.add)

                # -------- softmax (no max shift: |scores| small) --------
                p_sb = wp.tile([128, HW + 1], F32, tag="psb")
                sumexp = wp.tile([128, 1], F32, tag="sumexp")
                se2 = wp.tile([128, 1], F32, tag="se2")
                nc.scalar.activation(p_sb[:, 0:HW], sc_ps[:, 0:2, :], ACTF.Exp,
                                     scale=1.0, accum_out=sumexp[:])
                nc.scalar.activation(p_sb[:, HW:HW + 1], sc_ps[:, 2, 0:1],
                                     ACTF.Exp, scale=1.0, accum_out=se2[:])
                nc.vector.tensor_add(sumexp[:], sumexp[:], se2[:])
                rz = wp.tile([128, 1], F32, tag="rz")
                nc.vector.reciprocal(rz[:], sumexp[:])
                # p' : fold CLS-mean into token weights (in place, cols 0..1023)
                pcls_sc = wp.tile([128, 1], F32, tag="pclssc")
                nc.vector.tensor_scalar_mul(pcls_sc[:], p_sb[:, HW:HW + 1], 1.0 / HW)
                nc.vector.tensor_scalar_add(p_sb[:, 0:HW], p_sb[:, 0:HW], pcls_sc[:])

                # pT transposes (normalized via diag)
                pT = wp.tile([128, JT, 128], BF16, tag="pT")
                for half in range(2):
                    tp = psB.tile([128, 512], F32, tag="psB")
                    for k in range(4):
                        jc = half * 4 + k
                        nc.tensor.matmul(tp[:, ts(k, 128)],
                                         p_sb[:, ts(jc, 128)], identf_s[:],
                                         start=True, stop=True)
                    nc.vector.tensor_copy(
                        pT[:].rearrange("p j c -> p (j c)")[:, half * 512:(half + 1) * 512],
                        tp[:])
                pTc_ps = psB.tile([1, 128], F32, tag="psB")
                nc.tensor.matmul(pTc_ps[:], p_sb[:, HW:HW + 1], identf_s[:],
                                 start=True, stop=True)
                pTc = wp.tile([1, 128], BF16, tag="pTc")
                nc.vector.tensor_copy(pTc[:], pTc_ps[:])

                # ---------------- weighted sums ----------------
                w_ps = psD.tile([128, C], F32, tag="psD")
                for s in range(GRP):
                    for jc in range(JT):
                        nc.tensor.matmul(
                            w_ps[32 * s:32 * s + 32, :],
                            pT[:, jc, 32 * s:32 * s + 32], toks[s][:, :, jc, :],
                            start=(jc == 0), stop=(jc == JT - 1),
                            tile_position=(0, 32 * s))
                for jc in range(JT):
                    nc.tensor.matmul(w_ps[:], pT[:, jc, :], postok_s[:, jc, :],
                                     start=False, stop=False,
                                     skip_group_check=True)
                nc.tensor.matmul(w_ps[:], pTc[:], pos0row_s[:],
                                 start=False, stop=True, skip_group_check=True)
                w_sb = wp.tile([128, C], F32, tag="wsb")
                nc.vector.tensor_scalar_mul(w_sb[:], w_ps[:], rz[:])

                # wT via PE transpose
                tp3 = psB.tile([128, 512], F32, tag="psB")
                for mc in range(CT):
                    nc.tensor.transpose(tp3[:, ts(mc, 128)],
                                        w_sb[:, ts(mc, 128)], identf_s[:])
                wt_sb = wp.tile([128, CT, 128], F32, tag="wtsb")
                nc.vector.tensor_copy(wt_sb[:].rearrange("p t c -> p (t c)"), tp3[:])

                # ---------------- output projection ----------------
                out_ps = psC.tile([128, CT, GRP], F32, tag="psC")
                for h in range(NH):
                    pr, hi = h // 2, 64 * (h % 2)
                    for tk in range(CT):
                        nc.tensor.matmul(
                            out_ps[hi:hi + 64, pr, :],
                            wvt_s[:, tk, h * DH:(h + 1) * DH],
                            wt_sb[:, tk, h::32],
                            start=(tk == 0), stop=(tk == CT - 1),
                            tile_position=(0, hi),
                            skip_group_check=True)
                out_sb = wp.tile([128, CT, GRP], F32, tag="outsb")
                for pr in range(CT):
                    nc.vector.tensor_scalar_add(out_sb[:, pr, :],
                                                out_ps[:, pr, :],
                                                bv_s[:, pr:pr + 1])
                for s in range(GRP):
                    nc.sync.dma_start(
                        out=out_d[g * GRP + s].rearrange("(t p) -> p t", p=128),
                        in_=out_sb[:, :, s])

    nc.compile()
    return nc


def _prep(pos_emb, Wq, bq, Wk, bk, Wv, bv):
    # k/c dims live on partitions as c = p*CT + t (16KB-contiguous x loads)
    def ptn(v):  # [512] -> [128, CT], c = p*CT + t
        return np.ascontiguousarray(v.reshape(128, CT))

    def chunkk(w):  # [512, N] -> [128, CT, N], k = p*CT + t
        return np.ascontiguousarray(w.reshape(128, CT, -1))

    def chunkk_mperm(w):
        # rows k = p*CT+t on partitions; cols (the matmul M dim) permuted so
        # out partition p2 of chunk mc holds col index p2*CT + mc
        a = chunkk(w)                      # [128, CT, 512] cols natural
        a = a.reshape(128, CT, 128, CT).transpose(0, 1, 3, 2)  # [p,t,mc,p2]
        return np.ascontiguousarray(a.reshape(128, CT, C))

    order = np.r_[1:HW + 1, 0]
    kpm = Wk.astype(np.float64) @ pos_emb[order].astype(np.float64).T
    p1 = pos_emb[1:].sum(axis=0)
    pos0adj = pos_emb[0] - p1 / HW
    mask = np.zeros((128, CT, 32), np.float32)
    for p in range(128):
        for t in range(CT):
            h = (p * CT + t) // DH
            mask[p, t, h] = 1.0
    # w columns come out of the xbar-transposed tokens as (t, q): c = q*CT+t
    # -> permute pos token/cls rows to colflat = t*128 + q
    def wcols(v):  # [..., C] natural -> [..., C] in (t, q) order
        shp = v.shape[:-1]
        a = v.reshape(*shp, 128, CT)
        a = np.moveaxis(a, -1, -2)
        return np.ascontiguousarray(a.reshape(*shp, C))

    import ml_dtypes
    bf = ml_dtypes.bfloat16
    postok_nat = pos_emb[1:].reshape(JT, 128, C).transpose(1, 0, 2)
    return {
        "wqt": chunkk_mperm(np.ascontiguousarray(Wq.T)),
        "wk": chunkk_mperm(Wk).astype(bf),
        "wvt": chunkk(np.ascontiguousarray(Wv.T)),
        "kp": chunkk(kpm.astype(np.float32)).astype(bf),
        "postok": np.ascontiguousarray(wcols(postok_nat)).astype(bf),
        "pos0row": np.ascontiguousarray(wcols(pos0adj).reshape(1, C)).astype(bf),
        "pos0": ptn(pos_emb[0]),
        "bqs": ptn(bq * ISQ),
        "bv": np.ascontiguousarray(bv.reshape(CT, 128).T),
        "mask32": mask,
        "ident": np.eye(128, dtype=np.float32).astype(bf),
        "identf": np.eye(128, dtype=np.float32),
    }


def kernel(x, pos_emb, Wq, bq, Wk, bk, Wv, bv, num_heads):
    assert int(num_heads) == NH
    x = np.asarray(x, dtype=np.float32).reshape(B, C, HW)
    if "nc" not in _CACHE:
        _CACHE["nc"] = _build_nc()
    nc = _CACHE["nc"]
    shared = _prep(np.asarray(pos_emb, np.float32), np.asarray(Wq, np.float32),
                   np.asarray(bq, np.float32), np.asarray(Wk, np.float32),
                   np.asarray(bk, np.float32), np.asarray(Wv, np.float32),
                   np.asarray(bv, np.float32))
    in_maps = []
    for i in range(NCORES):
        m = dict(shared)
        m["xs"] = np.ascontiguousarray(x[i * BPC:(i + 1) * BPC])
        in_maps.append(m)
    res = run_bass_kernel_spmd(nc, in_maps, list(range(NCORES)))
    out = np.concatenate([res.results[i]["out"] for i in range(NCORES)], axis=0)
    return out.astype(np.float32)
